# revision 19
# baseline (speedup 1.0000x reference)
"""Single-head causal attention (B=8, T=2048, C=1024, H=128) on 8 trn2 cores.

Data-parallel over batch: core b computes attention for batch element b.

v6 changes vs v5 (trace-driven):
  - W stored cc-major; W+x0 posted as interleaved cc-pair chunks
    alternating rings, so QKV(0) can start ~10us and stream cc-major
    at the HBM feed rate (v5 waited until ~17us for x0)
  - QKV(0) emitted cc-major with q/k/v accumulating in 3 concurrent
    PSUM tiles; NWARM=6
  - o_ps double-buffered (pv(r+1) no longer waits outT(r) copy)
  - output DMAs split across both rings; l copy before ot3 copy so the
    two tail DMA desc-gens overlap
  - rest as v5: st(3) split across the middle, serial l-accumulator,
    batched pair exps, dedicated pre-zeroed diag tiles
"""

import numpy as np

import concourse.bacc as bacc
import concourse.mybir as mybir
import concourse.tile as tile
from concourse.bass_utils import run_bass_kernel_spmd

B, T, C, H = 8, 2048, 1024, 128
NCORES = 8
QR = 512          # q-range width (one PSUM bank)
NQR = T // QR     # 4 q-ranges
NKT = T // 128    # 16 k-strips
NCC = C // 128    # 8 contraction chunks
SCALE = 1.0 / np.sqrt(C)
NWARM = 6         # warmup matmuls (N=512)
NWARM2 = 6        # narrow warmup matmuls (N=128): fine-grained tail so the
                  # HAM activity window stays busy through DMA-arrival jitter

F32 = mybir.dt.float32
BF16 = mybir.dt.bfloat16
EXP = mybir.ActivationFunctionType.Exp


def _build_program():
    nc = bacc.Bacc("TRN2", target_bir_lowering=False, debug=False,
                   num_devices=NCORES)

    # x prepped as [s][128 p][cc][512]; W cc-major [128, cc, 3, H]
    x_d = nc.dram_tensor("x", [NQR, 128, NCC, QR], BF16, kind="ExternalInput")
    w_d = nc.dram_tensor("w", [128, 3 * NCC * H], BF16, kind="ExternalInput")
    mask_d = nc.dram_tensor("mask", [128, 128], BF16, kind="ExternalInput")
    out_d = nc.dram_tensor("out", [H, T], F32, kind="ExternalOutput")
    l_d = nc.dram_tensor("l", [1, T], F32, kind="ExternalOutput")

    with tile.TileContext(nc) as tc:
        with (
            tc.tile_pool(name="consts", bufs=1) as consts,
            tc.tile_pool(name="xt", bufs=NQR) as xt_pool,
            tc.tile_pool(name="qkvT", bufs=1) as qkvT_pool,
            tc.tile_pool(name="vnat", bufs=NQR) as vnat_pool,
            tc.tile_pool(name="e", bufs=12) as e_pool,
            tc.tile_pool(name="ediag", bufs=1) as ed_pool,
            tc.tile_pool(name="ep", bufs=8) as ep_pool,
            tc.tile_pool(name="lh", bufs=4) as lh_pool,
            tc.tile_pool(name="osmall", bufs=1) as osmall_pool,
            tc.tile_pool(name="mm", bufs=2, space="PSUM") as mm_pool,
            tc.tile_pool(name="st", bufs=2, space="PSUM") as st_pool,
            tc.tile_pool(name="oacc", bufs=2, space="PSUM") as oacc_pool,
        ):
            # ---- DMA posts FIRST so desc-gen starts at main() -------------
            w_sb = consts.tile([128, NCC, 3, H], BF16, tag="w")
            w_ap = w_d.ap().rearrange("p (cc w h) -> p cc w h", cc=NCC, w=3)
            xt = [xt_pool.tile([128, NCC, QR], BF16, tag="xt", name=f"xt{s}")
                  for s in range(NQR)]
            mask_sb = consts.tile([128, 128], BF16, tag="mask")

            # W+x0 cc-pairs interleaved, alternating rings, consumption order
            nc.scalar.dma_start(mask_sb[:], mask_d.ap())
            nc.sync.dma_start(w_sb[:, 0:2], w_ap[:, 0:2])
            nc.scalar.dma_start(w_sb[:, 2:4], w_ap[:, 2:4])
            nc.sync.dma_start(xt[0][:, 0:2, :], x_d.ap()[0, :, 0:2, :])
            nc.scalar.dma_start(xt[0][:, 2:4, :], x_d.ap()[0, :, 2:4, :])
            nc.sync.dma_start(w_sb[:, 4:6], w_ap[:, 4:6])
            nc.scalar.dma_start(w_sb[:, 6:8], w_ap[:, 6:8])
            nc.sync.dma_start(xt[0][:, 4:6, :], x_d.ap()[0, :, 4:6, :])
            nc.scalar.dma_start(xt[0][:, 6:8, :], x_d.ap()[0, :, 6:8, :])
            for s in (3, 1, 2):
                nc.sync.dma_start(xt[s][:, 0:4, :], x_d.ap()[s, :, 0:4, :])
                nc.scalar.dma_start(xt[s][:, 4:8, :], x_d.ap()[s, :, 4:8, :])

            # ---- local consts --------------------------------------------
            dummyw = consts.tile([128, 128], BF16, tag="dummyw")
            dummyx = consts.tile([128, QR], BF16, tag="dummyx")
            ones_sb = consts.tile([128, 1], BF16, tag="ones")
            nc.vector.memset(dummyw[:], 1.0)
            nc.vector.memset(dummyx[:], 0.0)
            nc.vector.memset(ones_sb[:], 1.0)

            # diag E tiles: dedicated per range, pre-zeroed once
            dA = [ed_pool.tile([128, 2, QR], BF16, tag=f"dA{r}",
                               name=f"dA{r}") for r in range(NQR)]
            dB = [ed_pool.tile([128, 2, QR], BF16, tag=f"dB{r}",
                               name=f"dB{r}") for r in range(NQR)]
            for t_ in dA + dB:
                nc.gpsimd.memset(t_[:], 0.0)

            # ---- PE warmup (into the o-acc bank) -------------------------
            warm_ps = oacc_pool.tile([128, QR], F32, tag="o")
            for _ in range(NWARM):
                nc.tensor.matmul(warm_ps[:], dummyw[:], dummyx[:],
                                 start=True, stop=True)
            for _ in range(NWARM2):
                nc.tensor.matmul(warm_ps[:, 0:128], dummyw[:],
                                 dummyx[:, 0:128], start=True, stop=True)
            nc.scalar.activation(dummyw[:, 0:1], dummyx[:, 0:1], EXP)

            # ---- qkv + v transpose ---------------------------------------
            qTs = [qkvT_pool.tile([128, QR], BF16, tag=f"qT{s}",
                                  name=f"qT{s}") for s in range(NQR)]
            kTs = [qkvT_pool.tile([128, QR], BF16, tag=f"kT{s}",
                                  name=f"kT{s}") for s in range(NQR)]
            vTs = [qkvT_pool.tile([128, QR], BF16, tag=f"vT{s}",
                                  name=f"vT{s}") for s in range(NQR)]
            vnat = [None] * NQR

            def kslice(kt):
                return kTs[kt // 4][:, 128 * (kt % 4):128 * (kt % 4 + 1)]

            def emit_qkv0_ccmajor():
                """QKV(0) cc-major: consume x0/W chunks as they land."""
                psq = mm_pool.tile([128, QR], F32, tag="mm")
                psk = mm_pool.tile([128, QR], F32, tag="mm")
                psv = oacc_pool.tile([128, QR], F32, tag="o")
                for cc in range(NCC):
                    for wi, ps in ((0, psq), (1, psk), (2, psv)):
                        nc.tensor.matmul(
                            ps[:], w_sb[:, cc, wi, :], xt[0][:, cc, :],
                            start=(cc == 0), stop=(cc == NCC - 1))
                # k copy on ACT so q and k land in parallel (st needs both)
                nc.vector.tensor_copy(qTs[0][:], psq[:])
                nc.scalar.copy(kTs[0][:], psk[:])
                nc.vector.tensor_copy(vTs[0][:], psv[:])

            def emit_qkv(s):
                for wi, dst in ((0, qTs[s]), (1, kTs[s]), (2, vTs[s])):
                    ps = mm_pool.tile([128, QR], F32, tag="mm")
                    for cc in range(NCC):
                        nc.tensor.matmul(
                            ps[:],
                            w_sb[:, cc, wi, :],
                            xt[s][:, cc, :],
                            start=(cc == 0), stop=(cc == NCC - 1))
                    if wi == 1:
                        nc.scalar.copy(dst[:], ps[:])
                    else:
                        nc.vector.tensor_copy(dst[:], ps[:])

            def emit_vtr(s):
                vt = vnat_pool.tile([128, 4, 128], BF16, tag="vnat",
                                    name=f"vnat{s}")
                nc.sync.dma_start_transpose(vt[:], vTs[s][:])
                vnat[s] = vt

            def vslice(kt):
                return vnat[kt // 4][:, kt % 4, :]

            # ---- attention ------------------------------------------------
            es_all = {r: [None] * (4 * r + 4) for r in range(NQR)}
            lacc_sb = {}
            lh_sb = {}

            def _lacc_add(r, pair_tile):
                prev = lacc_sb.get(r)
                if prev is None:
                    lacc_sb[r] = pair_tile
                else:
                    d = ep_pool.tile([128, 2, QR], BF16, tag="ep")
                    nc.vector.tensor_add(d[:], prev[:], pair_tile[:])
                    lacc_sb[r] = d

            def emit_lh(r):
                """Fold the range's accumulator to [128, QR]; eager so the
                l ones-matmul never waits at pv time."""
                root = lacc_sb[r]
                lh = lh_pool.tile([128, QR], BF16, tag="lh")
                nc.vector.tensor_add(lh[:], root[:, 0, :], root[:, 1, :])
                lh_sb[r] = lh

            def emit_st_full(r, pairs):
                for p in pairs:
                    st = st_pool.tile([128, 2, QR], F32, tag="st")
                    e = e_pool.tile([128, 2, QR], BF16, tag="e",
                                    name=f"e{r}_{p}")
                    for half in range(2):
                        kt = 2 * p + half
                        nc.tensor.matmul(
                            st[:, half, :], kslice(kt), qTs[r][:],
                            start=True, stop=True, skip_group_check=True)
                        es_all[r][kt] = (e, half, 0)
                    nc.scalar.activation(e[:], st[:], EXP, scale=float(SCALE))
                    _lacc_add(r, e)

            def emit_st_diag(r):
                for di, dt_ in ((0, dA[r]), (1, dB[r])):
                    st = st_pool.tile([128, 2, QR], F32, tag="st")
                    for half in range(2):
                        j = 2 * di + half
                        kt = 4 * r + j
                        off = 128 * j
                        nc.tensor.matmul(
                            st[:, half, off:QR], kslice(kt), qTs[r][:, off:QR],
                            start=True, stop=True, skip_group_check=True)
                        nc.scalar.activation(
                            dt_[:, half, off:QR], st[:, half, off:QR],
                            EXP, scale=float(SCALE))
                        nc.gpsimd.tensor_mul(
                            dt_[:, half, off:off + 128],
                            dt_[:, half, off:off + 128],
                            mask_sb[:])
                        es_all[r][kt] = (dt_, half, off)
                    _lacc_add(r, dt_)

            l_sb = osmall_pool.tile([1, T], F32, tag="l_sb")

            def emit_pv(r):
                nkt = 4 * r + 4
                es = es_all[r]
                # l before pv on PE: copy + l DMA overlap the pv matmuls
                l_ps = mm_pool.tile([1, QR], F32, tag="mm")
                nc.tensor.matmul(l_ps[:], ones_sb[:], lh_sb[r][:],
                                 start=True, stop=True)
                nc.scalar.copy(l_sb[:, QR * r:QR * (r + 1)], l_ps[:])
                if r == NQR - 1:
                    nc.sync.dma_start(l_d.ap()[:], l_sb[:])
                o_ps = oacc_pool.tile([128, QR], F32, tag="o")
                for kt in range(nkt):
                    e, half, off = es[kt]
                    nc.tensor.matmul(
                        o_ps[:, off:QR],
                        vslice(kt),
                        e[:, half, off:QR],
                        start=(kt == 0), stop=(kt == nkt - 1),
                        skip_group_check=True)
                ot = osmall_pool.tile([128, QR], F32, tag=f"outT{r}",
                                      name=f"ot{r}")
                if r == NQR - 1:
                    # tail: split the last outT copy across DVE+ACT and both
                    # rings so copy and desc-gen pipelines overlap
                    half = QR // 2
                    nc.vector.tensor_copy(ot[:, 0:half], o_ps[:, 0:half])
                    nc.scalar.copy(ot[:, half:QR], o_ps[:, half:QR])
                    nc.sync.dma_start(
                        out_d.ap()[:, QR * r:QR * r + half], ot[:, 0:half])
                    nc.scalar.dma_start(
                        out_d.ap()[:, QR * r + half:QR * (r + 1)],
                        ot[:, half:QR])
                else:
                    nc.vector.tensor_copy(ot[:], o_ps[:])
                    ring = nc.sync if r % 2 == 0 else nc.scalar
                    ring.dma_start(out_d.ap()[:, QR * r:QR * (r + 1)], ot[:])

            # ---- schedule -------------------------------------------------
            emit_qkv0_ccmajor()
            emit_vtr(0)
            emit_st_diag(0)
            emit_lh(0)
            emit_qkv(3)
            emit_vtr(3)
            emit_st_full(3, [0, 1])      # kt 0-3
            emit_st_diag(3)              # kt 12-15
            emit_qkv(1)
            emit_vtr(1)
            emit_st_full(1, [0, 1])
            emit_st_diag(1)
            emit_lh(1)
            emit_st_full(3, [2, 3])      # kt 4-7
            emit_qkv(2)
            emit_vtr(2)
            emit_st_full(2, [0, 1, 2, 3])
            emit_st_diag(2)
            emit_lh(2)
            emit_st_full(3, [4, 5])      # kt 8-11
            emit_lh(3)
            emit_pv(0)
            emit_pv(1)
            emit_pv(2)
            emit_pv(3)

    nc.compile()
    return nc


_PROGRAM = None


def _get_program():
    global _PROGRAM
    if _PROGRAM is None:
        _PROGRAM = _build_program()
    return _PROGRAM


import ml_dtypes

BF16_NP = ml_dtypes.bfloat16


def _host_inputs(x, Wq, Wk, Wv):
    x = np.asarray(x, dtype=np.float32)
    Wq = np.asarray(Wq, dtype=np.float32)
    Wk = np.asarray(Wk, dtype=np.float32)
    Wv = np.asarray(Wv, dtype=np.float32)

    p = np.arange(128)[:, None]
    f = np.arange(128)[None, :]
    mask = (f >= p).astype(BF16_NP)
    wstack = np.stack([Wq, Wk, Wv])  # [3, C, H]
    # cc-major: [128, cc, 3, H]
    wstack = wstack.reshape(3, NCC, 128, H).transpose(2, 1, 0, 3)
    wstack = np.ascontiguousarray(wstack.reshape(128, 3 * NCC * H)
                                  .astype(BF16_NP))

    in_maps = []
    for b in range(NCORES):
        xb = x[b].T.astype(BF16_NP)                       # [C, T]
        xb = xb.reshape(NCC, 128, NQR, QR).transpose(2, 1, 0, 3)
        in_maps.append({
            "x": np.ascontiguousarray(xb),
            "w": wstack, "mask": mask,
        })
    return in_maps


def run(x, Wq, Wk, Wv, trace=False, **kwargs):
    nc = _get_program()
    in_maps = _host_inputs(x, Wq, Wk, Wv)
    res = run_bass_kernel_spmd(nc, in_maps, core_ids=list(range(NCORES)),
                               trace=trace, **kwargs)
    outs = []
    for b in range(NCORES):
        oT = res.results[b]["out"].astype(np.float32)     # [H, T]
        l = res.results[b]["l"].astype(np.float32)        # [1, T]
        outs.append((oT / l).T)
    return np.stack(outs, axis=0).astype(np.float32), res


def kernel(x, Wq, Wk, Wv):
    out, _ = run(x, Wq, Wk, Wv)
    return out
